# revision 23
# baseline (speedup 1.0000x reference)
# GATv2 encoder (3x GATv2Conv, H=1) on 8 Trainium2 NeuronCores.
#
# Sharding: nodes partitioned by dst across 8 cores (graph parallel).
# Edge work per core is organized as bins of <=128 dst nodes (host-side
# bin-packing balances edge counts); a bin's edges are grouped by
# source-table chunk (4 chunks of 25088 rows so indices fit int16) and
# padded to 128-edge tiles.
#
# Per bin: batched dma_gather of source rows G = xl[src] (the ONLY
# Q7/SWDGE gather - xr[dst] is reconstructed on the TensorEngine as
# R = OT^T @ XRbin from host-shipped one-hot transposes), leaky-relu
# decomposed onto the Scalar engine, attention logits + softmax weights
# on Vector, and a single fused [num|den] matmul per tile accumulating
# in one PSUM bank.  Host gathers the per-core dense outputs between
# the three launches (all-gather of the xl tables).
import os
import sys
import math
import functools
import numpy as np

for _p in ("/opt/trn_rl_repo",):
    if _p not in sys.path and os.path.isdir(_p):
        sys.path.insert(0, _p)

import concourse.bass as bass
import concourse.mybir as mybir
import concourse.tile as tile
from concourse import bacc

F32 = mybir.dt.float32
BF16 = mybir.dt.bfloat16
F8 = mybir.dt.float8e4
I16 = mybir.dt.int16
I32 = mybir.dt.int32
AF = mybir.ActivationFunctionType
ALU = mybir.AluOpType

# Problem constants (hardcoded per contract)
N = 100_000
E = 1_600_000
IN, HID, OUT, H = 256, 128, 64, 1
SLOPE = 0.2
NCORES = 8
P = 128
EPS = 1e-30
NCHUNK = 4          # source-table chunks (rows per chunk must fit int16)
ZGRP = 4            # z tiles accumulated per PSUM bank


class Cfg:
    """Geometry, parameterized so small test instances can be built."""

    def __init__(self, n=N, e=E, fin=IN, hid=HID, out=OUT, ncores=NCORES):
        self.n, self.e, self.fin, self.hid, self.out = n, e, fin, hid, out
        self.ncores = ncores
        assert n % ncores == 0
        self.nl = n // ncores                  # dst nodes per core
        self.nbins = math.ceil(self.nl / P)    # bins per core
        self.nlp = self.nbins * P              # padded local nodes
        self.ntab = self.nlp * ncores          # rows in gathered tables
        # chunk bounds: full int16-reach chunks of 32768 rows + remainder
        self.cbounds = list(range(0, self.ntab, 32768)) + [self.ntab]
        self.nchunk = len(self.cbounds) - 1
        self.chsizes = [self.cbounds[i + 1] - self.cbounds[i]
                        for i in range(self.nchunk)]
        assert fin % P == 0
        self.kt = fin // P                     # K-tiles for dense1


# ----------------------------------------------------------------------------
# Host-side graph preprocessing
# ----------------------------------------------------------------------------

def prep_graph(cfg: Cfg, edge_index: np.ndarray):
    """Bin-pack dsts, group edges by (bin, src chunk), build index arrays."""
    n, ncores, nl, nbins, nlp = cfg.n, cfg.ncores, cfg.nl, cfg.nbins, cfg.nlp
    nck = cfg.nchunk + 1          # extra segment: self-loops from XLown
    cb = np.asarray(cfg.cbounds, dtype=np.int64)
    src = np.concatenate([edge_index[0], np.arange(n, dtype=np.int64)])
    dst = np.concatenate([edge_index[1], np.arange(n, dtype=np.int64)])
    is_self = np.zeros(src.shape[0], dtype=bool)
    is_self[edge_index.shape[1]:] = True

    # --- per-core bin-packing of dst nodes ---------------------------------
    slot_global = np.full(n, -1, dtype=np.int64)  # node -> row in table space
    deg_all = np.bincount(dst, minlength=n)
    import heapq
    for c in range(ncores):
        lo, hi = c * nl, (c + 1) * nl
        deg = deg_all[lo:hi]
        order = np.argsort(-deg, kind="stable")
        heap = [(0, 0, b) for b in range(nbins)]
        heapq.heapify(heap)
        stash = []
        for node in order:
            d = int(deg[node])
            while True:
                s, cnt, b = heapq.heappop(heap)
                if cnt < P:
                    break
                stash.append((s, cnt, b))
            slot_global[lo + node] = c * nlp + b * P + cnt
            heapq.heappush(heap, (s + d, cnt + 1, b))
            for it in stash:
                heapq.heappush(heap, it)
            stash.clear()

    # --- group edges by (core, bin, chunk) ---------------------------------
    sslot = slot_global[src]
    dslot = slot_global[dst]
    chunk = np.searchsorted(cb, sslot, side="right") - 1
    chunk[is_self] = nck - 1      # self-loops: local-table segment
    binid = dslot // P                    # global bin id = core*nbins + bin
    key = binid * nck + chunk
    order = np.argsort(key, kind="stable")
    s_o, d_o, k_o = sslot[order], dslot[order], key[order]
    nkeys = ncores * nbins * nck
    cnts = np.bincount(k_o, minlength=nkeys).reshape(ncores, nbins, nck)
    offs = np.concatenate([[0], np.cumsum(cnts.reshape(-1))])

    # uniform-across-cores idx-stream segments (16-granular) and tiles
    seg16 = ((cnts + 15) // 16 * 16).max(axis=0)  # [nbins, nck]
    tbo = ((seg16 + P - 1) // P).astype(np.int64)  # tiles per (bin, chunk)
    tbin = tbo.sum(axis=1)                 # [nbins] tiles per bin
    ntiles = int(tbin.sum())               # tiles per core (uniform)
    nslots = ntiles * P                    # edge slots per core
    nidx = int(seg16.sum())                # gather idx stream length

    # --- per-core index arrays ---------------------------------------------
    # gidx16: wrapped-16 int16 chunk-local src indices, [128, nslots//16]
    # Od:  [128, ntiles, 128] one-hot (edge-part, dstrow-free), fp8
    # OTd: [128, ntiles, 128] its transpose   (dstrow-part, edge-free)
    f8 = mybir.dt.np(F8)
    gidx16 = np.zeros((ncores, 128, nidx // 16), np.int16)
    Od = np.zeros((ncores, 128, ntiles, 128), f8)
    OTd = np.zeros((ncores, 128, ntiles, 128), f8)
    one = f8(1.0)
    for c in range(ncores):
        pos = 0   # slot position (tile-rounded per (bin, chunk))
        pix = 0   # idx-stream position (16-granular per (bin, chunk))
        for b in range(nbins):
            for o in range(nck):
                kk = int(cnts[c, b, o])
                so = offs[(c * nbins + b) * nck + o]
                seg = int(seg16[b, o])
                if seg == 0:
                    continue
                j = np.arange(kk)
                g = np.zeros(seg, np.int16)
                if o == nck - 1:
                    g[j] = (s_o[so:so + kk] % nlp).astype(np.int16)
                else:
                    g[j] = (s_o[so:so + kk] - cb[o]).astype(np.int16)
                jj = pix + np.arange(seg)
                gidx16[c, jj % 16, jj // 16] = g
                # one-hots for real edges only
                sj = pos + j
                r = (d_o[so:so + kk] - (c * nlp + b * P)).astype(np.int64)
                Od[c, sj % 128, sj // 128, r] = one
                OTd[c, r, sj // 128, sj % 128] = one
                pos += int(tbo[b, o]) * P
                pix += seg
        assert pos == nslots and pix == nidx
        # the Q7 gather ucode reads indices from its own 16-partition group:
        # replicate the wrapped-16 data across all 8 groups
        gidx16[c] = np.tile(gidx16[c, :16], (8, 1))

    # node permutation per core: slot s -> original node (or -1)
    perm = np.full((ncores, nlp), -1, dtype=np.int64)
    nodes = np.where(slot_global >= 0)[0]
    perm.reshape(-1)[slot_global[nodes]] = nodes

    return dict(
        tbo=tbo, tbin=tbin, seg16=seg16, nslots=nslots, ntiles=ntiles,
        nidx=nidx, slot_global=slot_global, perm=perm,
        gidx16=gidx16, Od=Od, OTd=OTd,
    )


# ----------------------------------------------------------------------------
# Device program builders (single SPMD program, data differs per core)
# ----------------------------------------------------------------------------

def _new_nc(cfg, nq=1):
    return bacc.Bacc("TRN2", target_bir_lowering=False, debug=False,
                     enable_asserts=False, num_devices=cfg.ncores,
                     num_swdge_queues=nq)


def build_dense1(cfg: Cfg, dt=F32):
    """xT [fin, nlp] -> XL1 [nlp, hid], XR1 [nlp, hid]."""
    nc = _new_nc(cfg)
    fin, hid, nlp, kt = cfg.fin, cfg.hid, cfg.nlp, cfg.kt
    xT = nc.dram_tensor("xT", [fin, nlp], BF16, kind="ExternalInput")
    wlr = nc.dram_tensor("wlr", [fin, 2 * hid], BF16, kind="ExternalInput")
    blrB = nc.dram_tensor("blrB", [P, 2 * hid], F32, kind="ExternalInput")
    XL = nc.dram_tensor("XL1", [nlp, hid], dt, kind="ExternalOutput")
    XR = nc.dram_tensor("XR1", [nlp, hid], dt, kind="ExternalOutput")

    mtiles = nlp // P
    with tile.TileContext(nc) as tc:
        with tc.tile_pool(name="const", bufs=1) as cp, \
             tc.tile_pool(name="work", bufs=4) as wp, \
             tc.tile_pool(name="psum", bufs=4, space="PSUM") as pp:
            xk = cp.tile([P, kt, nlp], BF16)
            nc.sync.dma_start(xk[:], xT[:].rearrange("(k p) n -> p k n", p=P))
            wlr_sb = cp.tile([P, kt, 2 * hid], BF16)
            nc.sync.dma_start(wlr_sb[:], wlr[:].rearrange("(k p) h -> p k h", p=P))
            blr_sb = cp.tile([P, 2 * hid], F32)
            nc.sync.dma_start(blr_sb[:], blrB[:])

            for m in range(mtiles):
                ms = slice(m * P, (m + 1) * P)
                pslr = pp.tile([P, 2 * hid], F32, tag="pslr")
                for k in range(kt):
                    nc.tensor.matmul(pslr[:], lhsT=xk[:, k, ms], rhs=wlr_sb[:, k, :],
                                     start=(k == 0), stop=(k == kt - 1))
                olr = wp.tile([P, 2 * hid], dt, tag="olr")
                nc.vector.tensor_tensor(out=olr[:], in0=pslr[:], in1=blr_sb[:],
                                        op=ALU.add)
                nc.sync.dma_start(XL[ms, :], olr[:, 0:hid])
                nc.sync.dma_start(XR[ms, :], olr[:, hid:2 * hid])
    nc.compile()
    return nc


def _edge_pipeline(nc, cfg, pr, pools, tabs, dt, per_bin):
    """Shared edge pipeline over bins.

    Per bin: chunked dma_gather of G = xl[src]; R = OT^T @ XRbin on the
    TensorEngine (ZGRP tiles share one PSUM bank); z = G + R batched on
    Vector; leaky relu decomposed on Scalar (0.8*relu(z) + 0.2*z); att
    logit mult on Vector; per_bin() finishes (reduce/exp/scale/matmul).
    """
    cp, gp, opl, wp, pp = pools
    XLchunks, XR, GIDX, Ot, OTt = tabs
    tbo, tbin, seg16 = pr["tbo"], pr["tbin"], pr["seg16"]
    nslots = pr["nslots"]
    nck = cfg.nchunk + 1
    Tmax = int(tbin.max())
    NQ = nc.num_swdge_queues
    qn = 0

    from concourse import library_config
    nc.gpsimd.load_library(library_config.mlp)

    # whole-launch index array resident in SBUF
    gix = cp.tile([P, pr["nidx"] // 16], I16)
    nc.sync.dma_start(gix[:], GIDX[:])

    # zero the gather buffers once: slots past a segment's end are never
    # written by any gather and must hold finite values
    for _ in range(5):
        Gz = gp.tile([P, Tmax, P], dt, tag="G", name="G")
        nc.vector.memset(Gz[:], 0.0)

    pix = 0
    tb = 0  # tile base
    for b in range(cfg.nbins):
        Tb = int(tbin[b])
        G = gp.tile([P, Tmax, P], dt, tag="G", name="G")
        to = 0
        for o in range(nck):
            tt = int(tbo[b, o])
            if tt == 0:
                continue
            seg = int(seg16[b, o])
            nc.gpsimd.dma_gather(
                out_ap=G[:, to:to + tt, :],
                in_ap=XLchunks[o][:],
                idxs_ap=gix[:, pix // 16:(pix + seg) // 16],
                num_idxs=seg, num_idxs_reg=seg, elem_size=P,
                single_packet=(seg <= 1024), queue_num=qn % NQ)
            qn += 1
            to += tt
            pix += seg
        assert to == Tb

        Osb = opl.tile([P, Tmax, P], F8, tag="O", name="O")
        nc.sync.dma_start(Osb[:, 0:Tb, :], Ot[:, tb:tb + Tb, :])
        OTsb = opl.tile([P, Tmax, P], F8, tag="OT", name="OT")
        nc.sync.dma_start(OTsb[:, 0:Tb, :], OTt[:, tb:tb + Tb, :])
        xrb = wp.tile([P, P], dt, tag="xrb", name="xrb")
        nc.sync.dma_start(xrb[:], XR[b * P:(b + 1) * P, :])

        # z = xr[dst] + G : R built on TensorE, PSUM->bf16 on Scalar,
        # then one batched bf16 add on Vector (2x perf mode)
        Rsb = wp.tile([P, Tmax, P], dt, tag="Rsb", name="Rsb")
        for g0 in range(0, Tb, ZGRP):
            gn = min(ZGRP, Tb - g0)
            zp = pp.tile([P, ZGRP, P], F32, tag="zp", name="zp")
            for j in range(gn):
                nc.tensor.matmul(zp[:, j, :], lhsT=OTsb[:, g0 + j, :], rhs=xrb[:],
                                 start=(j == 0), stop=(j == gn - 1),
                                 skip_group_check=True)
            nc.scalar.activation(out=Rsb[:, g0:g0 + gn, :], in_=zp[:, 0:gn, :],
                                 func=AF.Copy)
        zsb = wp.tile([P, Tmax, P], dt, tag="zsb", name="zsb")
        nc.vector.tensor_tensor(out=zsb[:, 0:Tb, :], in0=Rsb[:, 0:Tb, :],
                                in1=G[:, 0:Tb, :], op=ALU.add)

        # leaky_relu(z) = 0.8*relu(z) + 0.2*z, halves on the Scalar engine
        U1 = wp.tile([P, Tmax, P], dt, tag="U1", name="U1")
        nc.scalar.activation(out=U1[:, 0:Tb, :], in_=zsb[:, 0:Tb, :],
                             func=AF.Relu, scale=0.8)
        U2 = wp.tile([P, Tmax, P], dt, tag="U2", name="U2")
        nc.scalar.activation(out=U2[:, 0:Tb, :], in_=zsb[:, 0:Tb, :],
                             func=AF.Copy, scale=0.2)
        nc.vector.tensor_tensor(out=U1[:, 0:Tb, :], in0=U1[:, 0:Tb, :],
                                in1=U2[:, 0:Tb, :], op=ALU.add)

        per_bin(b, G, Osb, U1, Tb)
        tb += Tb
    assert pix == pr["nidx"]


def build_edge1(cfg: Cfg, pr, kpos, dt=BF16, nq=1):
    """Edge phase of layer 1 + dense transforms of layers 2/3.

    Attention vector is folded into the tables host-side: columns are
    permuted (positive-att features first, kpos of them) and scaled by
    |att|, so e = sum(U[:, :kpos]) - sum(U[:, kpos:]) and the aggregated
    values are unscaled by 1/|att| at finalize.
    """
    nc = _new_nc(cfg, nq)
    hid, nlp = cfg.hid, cfg.nlp
    nslots, ntiles = pr["nslots"], pr["ntiles"]
    Tmax = int(pr["tbin"].max())
    XLchunks = [nc.dram_tensor(f"XL1c{o}", [cfg.chsizes[o], hid], dt,
                               kind="ExternalInput") for o in range(cfg.nchunk)]
    XLchunks.append(nc.dram_tensor("XL1own", [nlp, hid], dt,
                                   kind="ExternalInput"))
    XR = nc.dram_tensor("XR1", [nlp, hid], dt, kind="ExternalInput")
    GIDX = nc.dram_tensor("gidx", [P, pr["nidx"] // 16], I16, kind="ExternalInput")
    Ot = nc.dram_tensor("Od", [P, ntiles, P], F8, kind="ExternalInput")
    OTt = nc.dram_tensor("OTd", [P, ntiles, P], F8, kind="ExternalInput")
    XL23 = nc.dram_tensor("XL23", [nlp, P], dt, kind="ExternalOutput")
    XR23 = nc.dram_tensor("XR23", [nlp, P], dt, kind="ExternalOutput")

    with tile.TileContext(nc) as tc:
        with tc.tile_pool(name="const", bufs=1) as cp, \
             tc.tile_pool(name="gath", bufs=5) as gp, \
             tc.tile_pool(name="onehot", bufs=3) as opl, \
             tc.tile_pool(name="work", bufs=4) as wp, \
             tc.tile_pool(name="psum", bufs=2, space="PSUM") as pp, \
             tc.tile_pool(name="psfin", bufs=2, space="PSUM") as pf:
            b1B_sb = cp.tile([P, hid], F32, name="b1B_sb")
            b1B = nc.dram_tensor("b1B", [P, hid], F32, kind="ExternalInput")
            nc.sync.dma_start(b1B_sb[:], b1B[:])
            ident_sb = cp.tile([P, P], F32, name="ident_sb")
            identB = nc.dram_tensor("identB", [P, P], F32, kind="ExternalInput")
            nc.sync.dma_start(ident_sb[:], identB[:])
            w23_sb = cp.tile([P, 2 * P], F32, name="w23_sb")
            w23 = nc.dram_tensor("w23lr", [hid, 2 * P], F32, kind="ExternalInput")
            nc.sync.dma_start(w23_sb[:], w23[:])
            b23_sb = cp.tile([P, 2 * P], F32, name="b23_sb")
            b23 = nc.dram_tensor("b23B", [P, 2 * P], F32, kind="ExternalInput")
            nc.sync.dma_start(b23_sb[:], b23[:])

            def per_bin(b, G, Osb, U1, Tb):
                ms = slice(b * P, (b + 1) * P)
                e = wp.tile([P, Tmax, 2, 1], F32, tag="e", name="e")
                nc.vector.tensor_reduce(
                    out=e[:, 0:Tb, 0:1, :],
                    in_=U1[:, 0:Tb, 0:kpos].rearrange("p t (l f) -> p t l f", l=1),
                    axis=mybir.AxisListType.X, op=ALU.add)
                nc.vector.tensor_reduce(
                    out=e[:, 0:Tb, 1:2, :],
                    in_=U1[:, 0:Tb, kpos:hid].rearrange("p t (l f) -> p t l f", l=1),
                    axis=mybir.AxisListType.X, op=ALU.add)
                es = wp.tile([P, Tmax, 1], F32, tag="es", name="es")
                nc.vector.tensor_tensor(out=es[:, 0:Tb, :], in0=e[:, 0:Tb, 0, :],
                                        in1=e[:, 0:Tb, 1, :], op=ALU.subtract)
                ee = wp.tile([P, Tmax, 1], F32, tag="ee", name="ee")
                nc.scalar.activation(out=ee[:, 0:Tb, :], in_=es[:, 0:Tb, :],
                                     func=AF.Exp)
                # Gp = [G * ee | ee]
                Gp = wp.tile([P, Tmax, P + 1], dt, tag="Gp", name="Gp")
                nc.vector.tensor_tensor(
                    out=Gp[:, 0:Tb, 0:P], in0=G[:, 0:Tb, :],
                    in1=ee[:, 0:Tb, :].to_broadcast([P, Tb, P]), op=ALU.mult)
                nc.vector.tensor_copy(Gp[:, 0:Tb, P:P + 1], ee[:, 0:Tb, :])
                # fused [num | den] accumulation
                pnd = pp.tile([P, P + 1], F32, tag="pnd", name="pnd")
                for t in range(Tb):
                    nc.tensor.matmul(pnd[:], lhsT=Osb[:, t, :],
                                     rhs=Gp[:, t, 0:P + 1],
                                     start=(t == 0), stop=(t == Tb - 1))
                # finalize: h = relu(num/den + b1); dense l/r of layers 2+3
                d = wp.tile([P, 1], F32, tag="d", name="d")
                nc.vector.tensor_scalar_add(d[:], pnd[:, P:P + 1], EPS)
                r = wp.tile([P, 1], F32, tag="r", name="r")
                nc.vector.reciprocal(r[:], d[:])
                h = wp.tile([P, hid], F32, tag="h", name="h")
                nc.scalar.activation(out=h[:], in_=pnd[:, 0:P],
                                     func=AF.Copy, scale=r[:])
                nc.vector.tensor_tensor(out=h[:], in0=h[:], in1=b1B_sb[:], op=ALU.add)
                nc.scalar.activation(out=h[:], in_=h[:], func=AF.Relu)
                pst = pf.tile([P, P], F32, tag="pst", name="pst")
                nc.tensor.transpose(out=pst[:], in_=h[:], identity=ident_sb[:])
                hT = wp.tile([P, P], F32, tag="hT", name="hT")
                nc.vector.tensor_copy(hT[:], pst[:])
                pslr = pf.tile([P, 2 * P], F32, tag="pslr", name="pslr")
                nc.tensor.matmul(pslr[:], lhsT=hT[:, 0:hid], rhs=w23_sb[:],
                                 start=True, stop=True)
                olr = wp.tile([P, 2 * P], dt, tag="olr", name="olr")
                nc.vector.tensor_tensor(out=olr[:], in0=pslr[:], in1=b23_sb[:],
                                        op=ALU.add)
                nc.sync.dma_start(XL23[ms, :], olr[:, 0:P])
                nc.sync.dma_start(XR23[ms, :], olr[:, P:2 * P])

            _edge_pipeline(nc, cfg, pr, (cp, gp, opl, wp, pp),
                           (XLchunks, XR, GIDX, Ot, OTt), dt, per_bin)
    nc.compile()
    return nc


def build_edge23(cfg: Cfg, pr, kposes, dt=BF16, nq=1):
    """Edge phases of layers 2 and 3 (shared gather)."""
    nc = _new_nc(cfg, nq)
    out, nlp = cfg.out, cfg.nlp
    nslots, ntiles = pr["nslots"], pr["ntiles"]
    Tmax = int(pr["tbin"].max())
    XLchunks = [nc.dram_tensor(f"XL23c{o}", [cfg.chsizes[o], P], dt,
                               kind="ExternalInput") for o in range(cfg.nchunk)]
    XLchunks.append(nc.dram_tensor("XL23own", [nlp, P], dt,
                                   kind="ExternalInput"))
    XR = nc.dram_tensor("XR23", [nlp, P], dt, kind="ExternalInput")
    GIDX = nc.dram_tensor("gidx", [P, pr["nidx"] // 16], I16, kind="ExternalInput")
    Ot = nc.dram_tensor("Od", [P, ntiles, P], F8, kind="ExternalInput")
    OTt = nc.dram_tensor("OTd", [P, ntiles, P], F8, kind="ExternalInput")
    MU = nc.dram_tensor("MU", [nlp, out], F32, kind="ExternalOutput")
    LV = nc.dram_tensor("LV", [nlp, out], F32, kind="ExternalOutput")

    with tile.TileContext(nc) as tc:
        with tc.tile_pool(name="const", bufs=1) as cp, \
             tc.tile_pool(name="gath", bufs=5) as gp, \
             tc.tile_pool(name="onehot", bufs=3) as opl, \
             tc.tile_pool(name="work", bufs=4) as wp, \
             tc.tile_pool(name="psum", bufs=3, space="PSUM") as pp:
            bmu_sb = cp.tile([P, out], F32, name="bmu_sb")
            bmuB = nc.dram_tensor("bmuB", [P, out], F32, kind="ExternalInput")
            nc.sync.dma_start(bmu_sb[:], bmuB[:])
            blv_sb = cp.tile([P, out], F32, name="blv_sb")
            blvB = nc.dram_tensor("blvB", [P, out], F32, kind="ExternalInput")
            nc.sync.dma_start(blv_sb[:], blvB[:])

            def per_bin(b, G, Osb, U1, Tb):
                ms = slice(b * P, (b + 1) * P)
                kmu, klv = kposes
                ep = wp.tile([P, Tmax, 2, 1], F32, tag="ep", name="ep")
                en = wp.tile([P, Tmax, 2, 1], F32, tag="en", name="en")
                for l, (lo, k) in enumerate(((0, kmu), (out, out + klv))):
                    nc.vector.tensor_reduce(
                        out=ep[:, 0:Tb, l:l + 1, :],
                        in_=U1[:, 0:Tb, lo:k].rearrange("p t (l f) -> p t l f", l=1),
                        axis=mybir.AxisListType.X, op=ALU.add)
                    nc.vector.tensor_reduce(
                        out=en[:, 0:Tb, l:l + 1, :],
                        in_=U1[:, 0:Tb, k:lo + out].rearrange("p t (l f) -> p t l f", l=1),
                        axis=mybir.AxisListType.X, op=ALU.add)
                e = wp.tile([P, Tmax, 2, 1], F32, tag="e", name="e")
                nc.vector.tensor_tensor(out=e[:, 0:Tb, :, :], in0=ep[:, 0:Tb, :, :],
                                        in1=en[:, 0:Tb, :, :], op=ALU.subtract)
                ee = wp.tile([P, Tmax, 2, 1], F32, tag="ee", name="ee")
                nc.scalar.activation(out=ee[:, 0:Tb, :, :], in_=e[:, 0:Tb, :, :],
                                     func=AF.Exp)
                # Gp = [G0*ee0 | G1*ee1 | ee0 | ee1]
                Gp = wp.tile([P, Tmax, P + 2], dt, tag="Gp", name="Gp")
                nc.vector.tensor_tensor(
                    out=Gp[:, 0:Tb, 0:P].rearrange("p t (l f) -> p t l f", l=2),
                    in0=G[:, 0:Tb, :].rearrange("p t (l f) -> p t l f", l=2),
                    in1=ee[:, 0:Tb, :, :].to_broadcast([P, Tb, 2, out]),
                    op=ALU.mult)
                nc.vector.tensor_copy(Gp[:, 0:Tb, P:P + 2],
                                      ee[:, 0:Tb, :, 0])
                pnd = pp.tile([P, P + 2], F32, tag="pnd", name="pnd")
                for t in range(Tb):
                    nc.tensor.matmul(pnd[:], lhsT=Osb[:, t, :],
                                     rhs=Gp[:, t, 0:P + 2],
                                     start=(t == 0), stop=(t == Tb - 1))
                for l, (bias, dest, tg) in enumerate(((bmu_sb, MU, "mu"),
                                                      (blv_sb, LV, "lv"))):
                    d = wp.tile([P, 1], F32, tag=f"d{tg}", name="d")
                    nc.vector.tensor_scalar_add(d[:], pnd[:, P + l:P + l + 1], EPS)
                    r = wp.tile([P, 1], F32, tag=f"r{tg}", name="r")
                    nc.vector.reciprocal(r[:], d[:])
                    o = wp.tile([P, out], F32, tag=f"o{tg}", name="o")
                    nc.scalar.activation(out=o[:], in_=pnd[:, l * out:(l + 1) * out],
                                         func=AF.Copy, scale=r[:])
                    nc.vector.tensor_tensor(out=o[:], in0=o[:], in1=bias[:], op=ALU.add)
                    nc.sync.dma_start(dest[ms, :], o[:])

            _edge_pipeline(nc, cfg, pr, (cp, gp, opl, wp, pp),
                           (XLchunks, XR, GIDX, Ot, OTt), dt, per_bin)
    nc.compile()
    return nc


# ----------------------------------------------------------------------------
# Host orchestration
# ----------------------------------------------------------------------------

def _bb(v, rows=P):
    """Broadcast a 1-D row vector to [rows, len] f32."""
    v = np.asarray(v, np.float32).reshape(1, -1)
    return np.ascontiguousarray(np.broadcast_to(v, (rows, v.shape[1])))


def _hw_runner(nc, in_maps, cfg, trace=False):
    from concourse import bass_utils
    r = bass_utils.run_bass_kernel_spmd(
        nc, in_maps, core_ids=list(range(cfg.ncores)), trace=trace)
    return r.results, r.exec_time_ns


class _State:
    """Cached compiled programs + prep, keyed by edge structure."""
    key = None
    progs = None
    prep = None


EDT = BF16 if not int(os.environ.get("GAT_F32", "0")) else F32
NQUEUES = 4


def build_progs(cfg, pr, kpos, dt=None, nq=None):
    dt = EDT if dt is None else dt
    nq = NQUEUES if nq is None else nq
    return dict(
        dense1=build_dense1(cfg, dt),
        edge1=build_edge1(cfg, pr, kpos[0], dt, nq),
        edge23=build_edge23(cfg, pr, (kpos[1], kpos[2]), dt, nq),
    )


def _att_fold(att):
    """Sign-permutation + |att| scale: positive-att features first."""
    att = att.reshape(-1).astype(np.float64)
    p = np.argsort(att < 0, kind="stable")
    k = int((att >= 0).sum())
    a = np.maximum(np.abs(att[p]), 1e-20)
    return p, k, a.astype(np.float32), (1.0 / a).astype(np.float32)


def forward(cfg, x, ei_unused, w, pr, progs, runner, dt=None):
    dt = EDT if dt is None else dt
    ndt = mybir.dt.np(dt)
    perm = pr["perm"]                    # [ncores, nlp] node ids or -1
    Tmax = int(pr["tbin"].max())
    profile = {}

    hid, out, nlp, ntab = cfg.hid, cfg.out, cfg.nlp, cfg.ntab
    bf = mybir.dt.np(BF16)
    p1, k1, a1, inv1 = _att_fold(w["sh_att"])
    pmu, kmu, amu, invmu = _att_fold(w["mu_att"])
    plv, klv, alv, invlv = _att_fold(w["lv_att"])

    # ---- launch A: dense1 (layer-1 att folded into the weights) -----------
    in_maps = []
    for c in range(cfg.ncores):
        xs = np.zeros((nlp, cfg.fin), np.float32)
        sel = perm[c] >= 0
        xs[sel] = x[perm[c][sel]]
        in_maps.append(dict(
            xT=np.ascontiguousarray(xs.T).astype(bf),
            wlr=np.ascontiguousarray(np.concatenate(
                [w["sh_Wl"][:, p1] * a1, w["sh_Wr"][:, p1] * a1],
                axis=1)).astype(bf),
            blrB=_bb(np.concatenate([w["sh_bl"][p1] * a1,
                                     w["sh_br"][p1] * a1]))))
    rA, profile["A"] = runner(progs["dense1"], in_maps, cfg)
    XL1full = np.concatenate([rA[c]["XL1"] for c in range(cfg.ncores)])
    XL1ch = {f"XL1c{o}": np.ascontiguousarray(
        XL1full[cfg.cbounds[o]:cfg.cbounds[o + 1]])
        for o in range(cfg.nchunk)}
    XR1 = [rA[c]["XR1"] for c in range(cfg.ncores)]

    # ---- launch B: edge1 + dense23 ----------------------------------------
    # w23 rows follow h's permuted hid order; columns carry layer-2/3 att
    # folding (sign-permute + |att| scale) so launch C's tables arrive folded.
    w23lr = inv1[:, None] * np.concatenate(
        [w["mu_Wl"][p1][:, pmu] * amu,
         w["lv_Wl"][p1][:, plv] * alv,
         w["mu_Wr"][p1][:, pmu] * amu,
         w["lv_Wr"][p1][:, plv] * alv], axis=1)
    b23 = np.concatenate([w["mu_bl"][pmu] * amu, w["lv_bl"][plv] * alv,
                          w["mu_br"][pmu] * amu, w["lv_br"][plv] * alv])
    ident = np.eye(P, dtype=np.float32)
    in_maps = []
    for c in range(cfg.ncores):
        in_maps.append(dict(
            XR1=XR1[c], **XL1ch, XL1own=rA[c]["XL1"],
            gidx=pr["gidx16"][c], Od=pr["Od"][c], OTd=pr["OTd"][c],
            b1B=_bb(w["sh_b"][p1] * a1), identB=ident,
            w23lr=np.ascontiguousarray(w23lr), b23B=_bb(b23)))
    rB, profile["B"] = runner(progs["edge1"], in_maps, cfg)
    XL23full = np.concatenate([rB[c]["XL23"] for c in range(cfg.ncores)])
    XL23ch = {f"XL23c{o}": np.ascontiguousarray(
        XL23full[cfg.cbounds[o]:cfg.cbounds[o + 1]])
        for o in range(cfg.nchunk)}
    XR23 = [rB[c]["XR23"] for c in range(cfg.ncores)]

    # ---- launch C: edge23 --------------------------------------------------
    in_maps = []
    for c in range(cfg.ncores):
        in_maps.append(dict(
            XR23=XR23[c], **XL23ch, XL23own=rB[c]["XL23"],
            gidx=pr["gidx16"][c], Od=pr["Od"][c], OTd=pr["OTd"][c],
            bmuB=_bb(w["mu_b"][pmu] * amu), blvB=_bb(w["lv_b"][plv] * alv)))
    rC, profile["C"] = runner(progs["edge23"], in_maps, cfg)

    MU = np.concatenate([rC[c]["MU"] for c in range(cfg.ncores)])
    LV = np.concatenate([rC[c]["LV"] for c in range(cfg.ncores)])
    mu = np.empty_like(MU)
    lv = np.empty_like(LV)
    mu[:, pmu] = MU * invmu  # undo sign permutation + |att| scale
    lv[:, plv] = LV * invlv
    mu = mu[pr["slot_global"]]
    lv = lv[pr["slot_global"]]
    return (mu, lv), profile


def kernel(**inputs):
    cfg = Cfg()
    x = np.asarray(inputs["x"], np.float32)
    ei = np.asarray(inputs["edge_index"]).astype(np.int64)
    w = {k: np.asarray(v, np.float32) for k, v in inputs.items()
         if k not in ("x", "edge_index")}

    kpos = tuple(int((w[f"{m}_att"].reshape(-1) >= 0).sum())
                 for m in ("sh", "mu", "lv"))
    key = hash(ei.tobytes() + bytes(kpos))
    if _State.key != key:
        pr = prep_graph(cfg, ei)
        _State.prep = pr
        _State.progs = build_progs(cfg, pr, kpos)
        _State.key = key

    trace = bool(int(os.environ.get("GAT_TRACE", "0")))
    runner = functools.partial(_hw_runner, trace=trace)
    (mu, lv), profile = forward(cfg, x, ei, w, _State.prep, _State.progs, runner)
    kernel._last_profile = profile
    return (mu, lv)


kernel._last_profile = None


# revision 25
# speedup vs baseline: 1.0237x; 1.0237x over previous
# GATv2 encoder (3x GATv2Conv, H=1) on 8 Trainium2 NeuronCores.
#
# Sharding: nodes partitioned by dst across 8 cores (graph parallel).
# Edge work per core is organized as bins of <=128 dst nodes (host-side
# bin-packing balances edge counts); a bin's edges are grouped by
# source-table chunk (4 chunks of 25088 rows so indices fit int16) and
# padded to 128-edge tiles.
#
# Per bin: batched dma_gather of source rows G = xl[src] (the ONLY
# Q7/SWDGE gather - xr[dst] is reconstructed on the TensorEngine as
# R = OT^T @ XRbin from host-shipped one-hot transposes), leaky-relu
# decomposed onto the Scalar engine, attention logits + softmax weights
# on Vector, and a single fused [num|den] matmul per tile accumulating
# in one PSUM bank.  Host gathers the per-core dense outputs between
# the three launches (all-gather of the xl tables).
import os
import sys
import math
import functools
import numpy as np

for _p in ("/opt/trn_rl_repo",):
    if _p not in sys.path and os.path.isdir(_p):
        sys.path.insert(0, _p)

import concourse.bass as bass
import concourse.mybir as mybir
import concourse.tile as tile
from concourse import bacc

F32 = mybir.dt.float32
BF16 = mybir.dt.bfloat16
F8 = mybir.dt.float8e4
I16 = mybir.dt.int16
I32 = mybir.dt.int32
AF = mybir.ActivationFunctionType
ALU = mybir.AluOpType

# Problem constants (hardcoded per contract)
N = 100_000
E = 1_600_000
IN, HID, OUT, H = 256, 128, 64, 1
SLOPE = 0.2
NCORES = 8
P = 128
EPS = 1e-30
NCHUNK = 4          # source-table chunks (rows per chunk must fit int16)
ZGRP = 4            # z tiles accumulated per PSUM bank


class Cfg:
    """Geometry, parameterized so small test instances can be built."""

    def __init__(self, n=N, e=E, fin=IN, hid=HID, out=OUT, ncores=NCORES):
        self.n, self.e, self.fin, self.hid, self.out = n, e, fin, hid, out
        self.ncores = ncores
        assert n % ncores == 0
        self.nl = n // ncores                  # dst nodes per core
        self.nbins = math.ceil(self.nl / P)    # bins per core
        self.nlp = self.nbins * P              # padded local nodes
        self.ntab = self.nlp * ncores          # rows in gathered tables
        # chunk bounds: full int16-reach chunks of 32768 rows + remainder
        self.cbounds = list(range(0, self.ntab, 32768)) + [self.ntab]
        self.nchunk = len(self.cbounds) - 1
        self.chsizes = [self.cbounds[i + 1] - self.cbounds[i]
                        for i in range(self.nchunk)]
        assert fin % P == 0
        self.kt = fin // P                     # K-tiles for dense1


# ----------------------------------------------------------------------------
# Host-side graph preprocessing
# ----------------------------------------------------------------------------

def prep_graph(cfg: Cfg, edge_index: np.ndarray):
    """Bin-pack dsts, group edges by (bin, src chunk), build index arrays."""
    n, ncores, nl, nbins, nlp = cfg.n, cfg.ncores, cfg.nl, cfg.nbins, cfg.nlp
    nck = cfg.nchunk + 1          # extra segment: self-loops from XLown
    cb = np.asarray(cfg.cbounds, dtype=np.int64)
    src = np.concatenate([edge_index[0], np.arange(n, dtype=np.int64)])
    dst = np.concatenate([edge_index[1], np.arange(n, dtype=np.int64)])
    is_self = np.zeros(src.shape[0], dtype=bool)
    is_self[edge_index.shape[1]:] = True

    # --- per-core bin-packing of dst nodes ---------------------------------
    slot_global = np.full(n, -1, dtype=np.int64)  # node -> row in table space
    deg_all = np.bincount(dst, minlength=n)
    import heapq
    for c in range(ncores):
        lo, hi = c * nl, (c + 1) * nl
        deg = deg_all[lo:hi]
        order = np.argsort(-deg, kind="stable")
        heap = [(0, 0, b) for b in range(nbins)]
        heapq.heapify(heap)
        stash = []
        for node in order:
            d = int(deg[node])
            while True:
                s, cnt, b = heapq.heappop(heap)
                if cnt < P:
                    break
                stash.append((s, cnt, b))
            slot_global[lo + node] = c * nlp + b * P + cnt
            heapq.heappush(heap, (s + d, cnt + 1, b))
            for it in stash:
                heapq.heappush(heap, it)
            stash.clear()

    # --- group edges by (core, bin, chunk) ---------------------------------
    sslot = slot_global[src]
    dslot = slot_global[dst]
    chunk = np.searchsorted(cb, sslot, side="right") - 1
    chunk[is_self] = nck - 1      # self-loops: local-table segment
    binid = dslot // P                    # global bin id = core*nbins + bin
    key = binid * nck + chunk
    order = np.argsort(key, kind="stable")
    s_o, d_o, k_o = sslot[order], dslot[order], key[order]
    nkeys = ncores * nbins * nck
    cnts = np.bincount(k_o, minlength=nkeys).reshape(ncores, nbins, nck)
    offs = np.concatenate([[0], np.cumsum(cnts.reshape(-1))])

    # uniform-across-cores idx-stream segments (16-granular) and tiles
    seg16 = ((cnts + 15) // 16 * 16).max(axis=0)  # [nbins, nck]
    tbo = ((seg16 + P - 1) // P).astype(np.int64)  # tiles per (bin, chunk)
    tbin = tbo.sum(axis=1)                 # [nbins] tiles per bin
    ntiles = int(tbin.sum())               # tiles per core (uniform)
    nslots = ntiles * P                    # edge slots per core
    nidx = int(seg16.sum())                # gather idx stream length

    # --- per-core index arrays ---------------------------------------------
    # gidx16: wrapped-16 int16 chunk-local src indices, [128, nslots//16]
    # Od:  [128, ntiles, 128] one-hot (edge-part, dstrow-free), fp8
    # OTd: [128, ntiles, 128] its transpose   (dstrow-part, edge-free)
    f8 = mybir.dt.np(F8)
    gidx16 = np.zeros((ncores, 128, nidx // 16), np.int16)
    Od = np.zeros((ncores, 128, ntiles, 128), f8)
    OTd = np.zeros((ncores, 128, ntiles, 128), f8)
    one = f8(1.0)
    for c in range(ncores):
        pos = 0   # slot position (tile-rounded per (bin, chunk))
        pix = 0   # idx-stream position (16-granular per (bin, chunk))
        for b in range(nbins):
            for o in range(nck):
                kk = int(cnts[c, b, o])
                so = offs[(c * nbins + b) * nck + o]
                seg = int(seg16[b, o])
                if seg == 0:
                    continue
                j = np.arange(kk)
                g = np.zeros(seg, np.int16)
                if o == nck - 1:
                    g[j] = (s_o[so:so + kk] % nlp).astype(np.int16)
                else:
                    g[j] = (s_o[so:so + kk] - cb[o]).astype(np.int16)
                jj = pix + np.arange(seg)
                gidx16[c, jj % 16, jj // 16] = g
                # one-hots for real edges only
                sj = pos + j
                r = (d_o[so:so + kk] - (c * nlp + b * P)).astype(np.int64)
                Od[c, sj % 128, sj // 128, r] = one
                OTd[c, r, sj // 128, sj % 128] = one
                pos += int(tbo[b, o]) * P
                pix += seg
        assert pos == nslots and pix == nidx
        # the Q7 gather ucode reads indices from its own 16-partition group:
        # replicate the wrapped-16 data across all 8 groups
        gidx16[c] = np.tile(gidx16[c, :16], (8, 1))

    # node permutation per core: slot s -> original node (or -1)
    perm = np.full((ncores, nlp), -1, dtype=np.int64)
    nodes = np.where(slot_global >= 0)[0]
    perm.reshape(-1)[slot_global[nodes]] = nodes

    return dict(
        tbo=tbo, tbin=tbin, seg16=seg16, nslots=nslots, ntiles=ntiles,
        nidx=nidx, slot_global=slot_global, perm=perm,
        gidx16=gidx16, Od=Od, OTd=OTd,
    )


# ----------------------------------------------------------------------------
# Device program builders (single SPMD program, data differs per core)
# ----------------------------------------------------------------------------

def _new_nc(cfg, nq=1):
    return bacc.Bacc("TRN2", target_bir_lowering=False, debug=False,
                     enable_asserts=False, num_devices=cfg.ncores,
                     num_swdge_queues=nq)


def build_dense1(cfg: Cfg, dt=F32):
    """xT [fin, nlp] -> XL1 [nlp, hid], XR1 [nlp, hid]."""
    nc = _new_nc(cfg)
    fin, hid, nlp, kt = cfg.fin, cfg.hid, cfg.nlp, cfg.kt
    xT = nc.dram_tensor("xT", [fin, nlp], BF16, kind="ExternalInput")
    wl = nc.dram_tensor("wl", [fin, hid], BF16, kind="ExternalInput")
    wr = nc.dram_tensor("wr", [fin, hid], BF16, kind="ExternalInput")
    blB = nc.dram_tensor("blB", [P, hid], F32, kind="ExternalInput")
    brB = nc.dram_tensor("brB", [P, hid], F32, kind="ExternalInput")
    XL = nc.dram_tensor("XL1", [nlp, hid], dt, kind="ExternalOutput")
    XR = nc.dram_tensor("XR1", [nlp, hid], dt, kind="ExternalOutput")

    mtiles = nlp // P
    with tile.TileContext(nc) as tc:
        with tc.tile_pool(name="const", bufs=1) as cp, \
             tc.tile_pool(name="work", bufs=4) as wp, \
             tc.tile_pool(name="psum", bufs=4, space="PSUM") as pp:
            xk = cp.tile([P, kt, nlp], BF16)
            nc.sync.dma_start(xk[:], xT[:].rearrange("(k p) n -> p k n", p=P))
            wl_sb = cp.tile([P, kt, hid], BF16)
            nc.sync.dma_start(wl_sb[:], wl[:].rearrange("(k p) h -> p k h", p=P))
            wr_sb = cp.tile([P, kt, hid], BF16)
            nc.sync.dma_start(wr_sb[:], wr[:].rearrange("(k p) h -> p k h", p=P))
            blB_sb = cp.tile([P, hid], F32)
            nc.sync.dma_start(blB_sb[:], blB[:])
            brB_sb = cp.tile([P, hid], F32)
            nc.sync.dma_start(brB_sb[:], brB[:])

            for m in range(mtiles):
                ms = slice(m * P, (m + 1) * P)
                psl = pp.tile([P, hid], F32, tag="psl")
                psr = pp.tile([P, hid], F32, tag="psr")
                for k in range(kt):
                    nc.tensor.matmul(psl[:], lhsT=xk[:, k, ms], rhs=wl_sb[:, k, :],
                                     start=(k == 0), stop=(k == kt - 1))
                for k in range(kt):
                    nc.tensor.matmul(psr[:], lhsT=xk[:, k, ms], rhs=wr_sb[:, k, :],
                                     start=(k == 0), stop=(k == kt - 1))
                ol = wp.tile([P, hid], dt, tag="ol")
                nc.vector.tensor_tensor(out=ol[:], in0=psl[:], in1=blB_sb[:], op=ALU.add)
                orr = wp.tile([P, hid], dt, tag="orr")
                nc.vector.tensor_tensor(out=orr[:], in0=psr[:], in1=brB_sb[:], op=ALU.add)
                nc.sync.dma_start(XL[ms, :], ol[:])
                nc.sync.dma_start(XR[ms, :], orr[:])
    nc.compile()
    return nc


def _edge_pipeline(nc, cfg, pr, pools, tabs, dt, per_bin):
    """Shared edge pipeline over bins.

    Per bin: chunked dma_gather of G = xl[src]; R = OT^T @ XRbin on the
    TensorEngine (ZGRP tiles share one PSUM bank); z = G + R batched on
    Vector; leaky relu decomposed on Scalar (0.8*relu(z) + 0.2*z); att
    logit mult on Vector; per_bin() finishes (reduce/exp/scale/matmul).
    """
    cp, gp, opl, wp, pp = pools
    XLchunks, XR, GIDX, Ot, OTt = tabs
    tbo, tbin, seg16 = pr["tbo"], pr["tbin"], pr["seg16"]
    nslots = pr["nslots"]
    nck = cfg.nchunk + 1
    Tmax = int(tbin.max())
    NQ = nc.num_swdge_queues
    qn = 0

    from concourse import library_config
    nc.gpsimd.load_library(library_config.mlp)

    # whole-launch index array resident in SBUF
    gix = cp.tile([P, pr["nidx"] // 16], I16)
    nc.sync.dma_start(gix[:], GIDX[:])

    # zero the gather buffers once: slots past a segment's end are never
    # written by any gather and must hold finite values
    for _ in range(5):
        Gz = gp.tile([P, Tmax, P], dt, tag="G", name="G")
        nc.vector.memset(Gz[:], 0.0)

    pix = 0
    tb = 0  # tile base
    for b in range(cfg.nbins):
        Tb = int(tbin[b])
        G = gp.tile([P, Tmax, P], dt, tag="G", name="G")
        to = 0
        for o in range(nck):
            tt = int(tbo[b, o])
            if tt == 0:
                continue
            seg = int(seg16[b, o])
            nc.gpsimd.dma_gather(
                out_ap=G[:, to:to + tt, :],
                in_ap=XLchunks[o][:],
                idxs_ap=gix[:, pix // 16:(pix + seg) // 16],
                num_idxs=seg, num_idxs_reg=seg, elem_size=P,
                single_packet=(seg <= 1024), queue_num=qn % NQ)
            qn += 1
            to += tt
            pix += seg
        assert to == Tb

        Osb = opl.tile([P, Tmax, P], F8, tag="O", name="O")
        nc.sync.dma_start(Osb[:, 0:Tb, :], Ot[:, tb:tb + Tb, :])
        OTsb = opl.tile([P, Tmax, P], F8, tag="OT", name="OT")
        nc.sync.dma_start(OTsb[:, 0:Tb, :], OTt[:, tb:tb + Tb, :])
        xrb = wp.tile([P, P], dt, tag="xrb", name="xrb")
        nc.sync.dma_start(xrb[:], XR[b * P:(b + 1) * P, :])

        # z = xr[dst] + G : R built on TensorE, PSUM->bf16 on Scalar,
        # then one batched bf16 add on Vector (2x perf mode)
        Rsb = wp.tile([P, Tmax, P], dt, tag="Rsb", name="Rsb")
        for g0 in range(0, Tb, ZGRP):
            gn = min(ZGRP, Tb - g0)
            zp = pp.tile([P, ZGRP, P], F32, tag="zp", name="zp")
            for j in range(gn):
                nc.tensor.matmul(zp[:, j, :], lhsT=OTsb[:, g0 + j, :], rhs=xrb[:],
                                 start=(j == 0), stop=(j == gn - 1),
                                 skip_group_check=True)
            nc.scalar.activation(out=Rsb[:, g0:g0 + gn, :], in_=zp[:, 0:gn, :],
                                 func=AF.Copy)
        zsb = wp.tile([P, Tmax, P], dt, tag="zsb", name="zsb")
        nc.vector.tensor_tensor(out=zsb[:, 0:Tb, :], in0=Rsb[:, 0:Tb, :],
                                in1=G[:, 0:Tb, :], op=ALU.add)

        # leaky_relu(z) = 0.8*relu(z) + 0.2*z, halves on the Scalar engine
        U1 = wp.tile([P, Tmax, P], dt, tag="U1", name="U1")
        nc.scalar.activation(out=U1[:, 0:Tb, :], in_=zsb[:, 0:Tb, :],
                             func=AF.Relu, scale=0.8)
        U2 = wp.tile([P, Tmax, P], dt, tag="U2", name="U2")
        nc.scalar.activation(out=U2[:, 0:Tb, :], in_=zsb[:, 0:Tb, :],
                             func=AF.Copy, scale=0.2)
        nc.vector.tensor_tensor(out=U1[:, 0:Tb, :], in0=U1[:, 0:Tb, :],
                                in1=U2[:, 0:Tb, :], op=ALU.add)

        per_bin(b, G, Osb, U1, Tb)
        tb += Tb
    assert pix == pr["nidx"]


def build_edge1(cfg: Cfg, pr, kpos, dt=BF16, nq=1):
    """Edge phase of layer 1 + dense transforms of layers 2/3.

    Attention vector is folded into the tables host-side: columns are
    permuted (positive-att features first, kpos of them) and scaled by
    |att|, so e = sum(U[:, :kpos]) - sum(U[:, kpos:]) and the aggregated
    values are unscaled by 1/|att| at finalize.
    """
    nc = _new_nc(cfg, nq)
    hid, nlp = cfg.hid, cfg.nlp
    nslots, ntiles = pr["nslots"], pr["ntiles"]
    Tmax = int(pr["tbin"].max())
    XLchunks = [nc.dram_tensor(f"XL1c{o}", [cfg.chsizes[o], hid], dt,
                               kind="ExternalInput") for o in range(cfg.nchunk)]
    XLchunks.append(nc.dram_tensor("XL1own", [nlp, hid], dt,
                                   kind="ExternalInput"))
    XR = nc.dram_tensor("XR1", [nlp, hid], dt, kind="ExternalInput")
    GIDX = nc.dram_tensor("gidx", [P, pr["nidx"] // 16], I16, kind="ExternalInput")
    Ot = nc.dram_tensor("Od", [P, ntiles, P], F8, kind="ExternalInput")
    OTt = nc.dram_tensor("OTd", [P, ntiles, P], F8, kind="ExternalInput")
    XL23 = nc.dram_tensor("XL23", [nlp, P], dt, kind="ExternalOutput")
    XR23 = nc.dram_tensor("XR23", [nlp, P], dt, kind="ExternalOutput")

    with tile.TileContext(nc) as tc:
        with tc.tile_pool(name="const", bufs=1) as cp, \
             tc.tile_pool(name="gath", bufs=5) as gp, \
             tc.tile_pool(name="onehot", bufs=3) as opl, \
             tc.tile_pool(name="work", bufs=4) as wp, \
             tc.tile_pool(name="psum", bufs=2, space="PSUM") as pp, \
             tc.tile_pool(name="psfin", bufs=2, space="PSUM") as pf:
            inv1_sb = cp.tile([P, hid], F32, name="inv1_sb")
            inv1B = nc.dram_tensor("inv1B", [P, hid], F32, kind="ExternalInput")
            nc.sync.dma_start(inv1_sb[:], inv1B[:])
            b1B_sb = cp.tile([P, hid], F32, name="b1B_sb")
            b1B = nc.dram_tensor("b1B", [P, hid], F32, kind="ExternalInput")
            nc.sync.dma_start(b1B_sb[:], b1B[:])
            ident_sb = cp.tile([P, P], F32, name="ident_sb")
            identB = nc.dram_tensor("identB", [P, P], F32, kind="ExternalInput")
            nc.sync.dma_start(ident_sb[:], identB[:])
            w23_sb = cp.tile([P, 2 * P], F32, name="w23_sb")
            w23 = nc.dram_tensor("w23lr", [hid, 2 * P], F32, kind="ExternalInput")
            nc.sync.dma_start(w23_sb[:], w23[:])
            b23_sb = cp.tile([P, 2 * P], F32, name="b23_sb")
            b23 = nc.dram_tensor("b23B", [P, 2 * P], F32, kind="ExternalInput")
            nc.sync.dma_start(b23_sb[:], b23[:])

            def per_bin(b, G, Osb, U1, Tb):
                ms = slice(b * P, (b + 1) * P)
                e = wp.tile([P, Tmax, 2, 1], F32, tag="e", name="e")
                nc.vector.tensor_reduce(
                    out=e[:, 0:Tb, 0:1, :],
                    in_=U1[:, 0:Tb, 0:kpos].rearrange("p t (l f) -> p t l f", l=1),
                    axis=mybir.AxisListType.X, op=ALU.add)
                nc.vector.tensor_reduce(
                    out=e[:, 0:Tb, 1:2, :],
                    in_=U1[:, 0:Tb, kpos:hid].rearrange("p t (l f) -> p t l f", l=1),
                    axis=mybir.AxisListType.X, op=ALU.add)
                es = wp.tile([P, Tmax, 1], F32, tag="es", name="es")
                nc.vector.tensor_tensor(out=es[:, 0:Tb, :], in0=e[:, 0:Tb, 0, :],
                                        in1=e[:, 0:Tb, 1, :], op=ALU.subtract)
                ee = wp.tile([P, Tmax, 1], F32, tag="ee", name="ee")
                nc.scalar.activation(out=ee[:, 0:Tb, :], in_=es[:, 0:Tb, :],
                                     func=AF.Exp)
                # Gp = [G * ee | ee]
                Gp = wp.tile([P, Tmax, P + 1], dt, tag="Gp", name="Gp")
                nc.vector.tensor_tensor(
                    out=Gp[:, 0:Tb, 0:P], in0=G[:, 0:Tb, :],
                    in1=ee[:, 0:Tb, :].to_broadcast([P, Tb, P]), op=ALU.mult)
                nc.vector.tensor_copy(Gp[:, 0:Tb, P:P + 1], ee[:, 0:Tb, :])
                # fused [num | den] accumulation
                pnd = pp.tile([P, P + 1], F32, tag="pnd", name="pnd")
                for t in range(Tb):
                    nc.tensor.matmul(pnd[:], lhsT=Osb[:, t, :],
                                     rhs=Gp[:, t, 0:P + 1],
                                     start=(t == 0), stop=(t == Tb - 1))
                # finalize: h = relu(num/den + b1); dense l/r of layers 2+3
                d = wp.tile([P, 1], F32, tag="d", name="d")
                nc.vector.tensor_scalar_add(d[:], pnd[:, P:P + 1], EPS)
                r = wp.tile([P, 1], F32, tag="r", name="r")
                nc.vector.reciprocal(r[:], d[:])
                h = wp.tile([P, hid], F32, tag="h", name="h")
                nc.scalar.activation(out=h[:], in_=pnd[:, 0:P],
                                     func=AF.Copy, scale=r[:])
                nc.vector.tensor_tensor(out=h[:], in0=h[:], in1=inv1_sb[:], op=ALU.mult)
                nc.vector.tensor_tensor(out=h[:], in0=h[:], in1=b1B_sb[:], op=ALU.add)
                nc.scalar.activation(out=h[:], in_=h[:], func=AF.Relu)
                pst = pf.tile([P, P], F32, tag="pst", name="pst")
                nc.tensor.transpose(out=pst[:], in_=h[:], identity=ident_sb[:])
                hT = wp.tile([P, P], F32, tag="hT", name="hT")
                nc.vector.tensor_copy(hT[:], pst[:])
                pslr = pf.tile([P, 2 * P], F32, tag="pslr", name="pslr")
                nc.tensor.matmul(pslr[:], lhsT=hT[:, 0:hid], rhs=w23_sb[:],
                                 start=True, stop=True)
                olr = wp.tile([P, 2 * P], dt, tag="olr", name="olr")
                nc.vector.tensor_tensor(out=olr[:], in0=pslr[:], in1=b23_sb[:],
                                        op=ALU.add)
                nc.sync.dma_start(XL23[ms, :], olr[:, 0:P])
                nc.sync.dma_start(XR23[ms, :], olr[:, P:2 * P])

            _edge_pipeline(nc, cfg, pr, (cp, gp, opl, wp, pp),
                           (XLchunks, XR, GIDX, Ot, OTt), dt, per_bin)
    nc.compile()
    return nc


def build_edge23(cfg: Cfg, pr, kposes, dt=BF16, nq=1):
    """Edge phases of layers 2 and 3 (shared gather)."""
    nc = _new_nc(cfg, nq)
    out, nlp = cfg.out, cfg.nlp
    nslots, ntiles = pr["nslots"], pr["ntiles"]
    Tmax = int(pr["tbin"].max())
    XLchunks = [nc.dram_tensor(f"XL23c{o}", [cfg.chsizes[o], P], dt,
                               kind="ExternalInput") for o in range(cfg.nchunk)]
    XLchunks.append(nc.dram_tensor("XL23own", [nlp, P], dt,
                                   kind="ExternalInput"))
    XR = nc.dram_tensor("XR23", [nlp, P], dt, kind="ExternalInput")
    GIDX = nc.dram_tensor("gidx", [P, pr["nidx"] // 16], I16, kind="ExternalInput")
    Ot = nc.dram_tensor("Od", [P, ntiles, P], F8, kind="ExternalInput")
    OTt = nc.dram_tensor("OTd", [P, ntiles, P], F8, kind="ExternalInput")
    MU = nc.dram_tensor("MU", [nlp, out], F32, kind="ExternalOutput")
    LV = nc.dram_tensor("LV", [nlp, out], F32, kind="ExternalOutput")

    with tile.TileContext(nc) as tc:
        with tc.tile_pool(name="const", bufs=1) as cp, \
             tc.tile_pool(name="gath", bufs=5) as gp, \
             tc.tile_pool(name="onehot", bufs=3) as opl, \
             tc.tile_pool(name="work", bufs=4) as wp, \
             tc.tile_pool(name="psum", bufs=4, space="PSUM") as pp:
            invmu_sb = cp.tile([P, out], F32, name="invmu_sb")
            invmuB = nc.dram_tensor("invmuB", [P, out], F32, kind="ExternalInput")
            nc.sync.dma_start(invmu_sb[:], invmuB[:])
            invlv_sb = cp.tile([P, out], F32, name="invlv_sb")
            invlvB = nc.dram_tensor("invlvB", [P, out], F32, kind="ExternalInput")
            nc.sync.dma_start(invlv_sb[:], invlvB[:])
            bmu_sb = cp.tile([P, out], F32, name="bmu_sb")
            bmuB = nc.dram_tensor("bmuB", [P, out], F32, kind="ExternalInput")
            nc.sync.dma_start(bmu_sb[:], bmuB[:])
            blv_sb = cp.tile([P, out], F32, name="blv_sb")
            blvB = nc.dram_tensor("blvB", [P, out], F32, kind="ExternalInput")
            nc.sync.dma_start(blv_sb[:], blvB[:])

            def per_bin(b, G, Osb, U1, Tb):
                ms = slice(b * P, (b + 1) * P)
                kmu, klv = kposes
                ep = wp.tile([P, Tmax, 2, 1], F32, tag="ep", name="ep")
                en = wp.tile([P, Tmax, 2, 1], F32, tag="en", name="en")
                for l, (lo, k) in enumerate(((0, kmu), (out, out + klv))):
                    nc.vector.tensor_reduce(
                        out=ep[:, 0:Tb, l:l + 1, :],
                        in_=U1[:, 0:Tb, lo:k].rearrange("p t (l f) -> p t l f", l=1),
                        axis=mybir.AxisListType.X, op=ALU.add)
                    nc.vector.tensor_reduce(
                        out=en[:, 0:Tb, l:l + 1, :],
                        in_=U1[:, 0:Tb, k:lo + out].rearrange("p t (l f) -> p t l f", l=1),
                        axis=mybir.AxisListType.X, op=ALU.add)
                e = wp.tile([P, Tmax, 2, 1], F32, tag="e", name="e")
                nc.vector.tensor_tensor(out=e[:, 0:Tb, :, :], in0=ep[:, 0:Tb, :, :],
                                        in1=en[:, 0:Tb, :, :], op=ALU.subtract)
                ee = wp.tile([P, Tmax, 2, 1], F32, tag="ee", name="ee")
                nc.scalar.activation(out=ee[:, 0:Tb, :, :], in_=e[:, 0:Tb, :, :],
                                     func=AF.Exp)
                # Gp = [G0*ee0 | G1*ee1 | ee0 | ee1]
                Gp = wp.tile([P, Tmax, P + 2], dt, tag="Gp", name="Gp")
                nc.vector.tensor_tensor(
                    out=Gp[:, 0:Tb, 0:P].rearrange("p t (l f) -> p t l f", l=2),
                    in0=G[:, 0:Tb, :].rearrange("p t (l f) -> p t l f", l=2),
                    in1=ee[:, 0:Tb, :, :].to_broadcast([P, Tb, 2, out]),
                    op=ALU.mult)
                nc.vector.tensor_copy(Gp[:, 0:Tb, P:P + 2],
                                      ee[:, 0:Tb, :, 0])
                pnd = pp.tile([P, P + 2], F32, tag="pnd", name="pnd")
                for t in range(Tb):
                    nc.tensor.matmul(pnd[:], lhsT=Osb[:, t, :],
                                     rhs=Gp[:, t, 0:P + 2],
                                     start=(t == 0), stop=(t == Tb - 1))
                for l, (inv, bias, dest, tg) in enumerate((
                        (invmu_sb, bmu_sb, MU, "mu"),
                        (invlv_sb, blv_sb, LV, "lv"))):
                    d = wp.tile([P, 1], F32, tag=f"d{tg}", name="d")
                    nc.vector.tensor_scalar_add(d[:], pnd[:, P + l:P + l + 1], EPS)
                    r = wp.tile([P, 1], F32, tag=f"r{tg}", name="r")
                    nc.vector.reciprocal(r[:], d[:])
                    o = wp.tile([P, out], F32, tag=f"o{tg}", name="o")
                    nc.scalar.activation(out=o[:], in_=pnd[:, l * out:(l + 1) * out],
                                         func=AF.Copy, scale=r[:])
                    nc.vector.tensor_tensor(out=o[:], in0=o[:], in1=inv[:], op=ALU.mult)
                    nc.vector.tensor_tensor(out=o[:], in0=o[:], in1=bias[:], op=ALU.add)
                    nc.sync.dma_start(dest[ms, :], o[:])

            _edge_pipeline(nc, cfg, pr, (cp, gp, opl, wp, pp),
                           (XLchunks, XR, GIDX, Ot, OTt), dt, per_bin)
    nc.compile()
    return nc


# ----------------------------------------------------------------------------
# Host orchestration
# ----------------------------------------------------------------------------

def _bb(v, rows=P):
    """Broadcast a 1-D row vector to [rows, len] f32."""
    v = np.asarray(v, np.float32).reshape(1, -1)
    return np.ascontiguousarray(np.broadcast_to(v, (rows, v.shape[1])))


def _hw_runner(nc, in_maps, cfg, trace=False):
    from concourse import bass_utils
    r = bass_utils.run_bass_kernel_spmd(
        nc, in_maps, core_ids=list(range(cfg.ncores)), trace=trace)
    return r.results, r.exec_time_ns


class _State:
    """Cached compiled programs + prep, keyed by edge structure."""
    key = None
    progs = None
    prep = None


EDT = BF16 if not int(os.environ.get("GAT_F32", "0")) else F32
NQUEUES = 4


def build_progs(cfg, pr, kpos, dt=None, nq=None):
    dt = EDT if dt is None else dt
    nq = NQUEUES if nq is None else nq
    return dict(
        dense1=build_dense1(cfg, dt),
        edge1=build_edge1(cfg, pr, kpos[0], dt, nq),
        edge23=build_edge23(cfg, pr, (kpos[1], kpos[2]), dt, nq),
    )


def _att_fold(att):
    """Sign-permutation + |att| scale: positive-att features first."""
    att = att.reshape(-1).astype(np.float64)
    p = np.argsort(att < 0, kind="stable")
    k = int((att >= 0).sum())
    a = np.maximum(np.abs(att[p]), 1e-20)
    return p, k, a.astype(np.float32), (1.0 / a).astype(np.float32)


def forward(cfg, x, ei_unused, w, pr, progs, runner, dt=None):
    dt = EDT if dt is None else dt
    ndt = mybir.dt.np(dt)
    perm = pr["perm"]                    # [ncores, nlp] node ids or -1
    Tmax = int(pr["tbin"].max())
    profile = {}

    hid, out, nlp, ntab = cfg.hid, cfg.out, cfg.nlp, cfg.ntab
    bf = mybir.dt.np(BF16)
    p1, k1, a1, inv1 = _att_fold(w["sh_att"])
    pmu, kmu, amu, invmu = _att_fold(w["mu_att"])
    plv, klv, alv, invlv = _att_fold(w["lv_att"])

    # ---- launch A: dense1 (layer-1 att folded into the weights) -----------
    in_maps = []
    for c in range(cfg.ncores):
        xs = np.zeros((nlp, cfg.fin), np.float32)
        sel = perm[c] >= 0
        xs[sel] = x[perm[c][sel]]
        in_maps.append(dict(
            xT=np.ascontiguousarray(xs.T).astype(bf),
            wl=np.ascontiguousarray(w["sh_Wl"][:, p1] * a1).astype(bf),
            wr=np.ascontiguousarray(w["sh_Wr"][:, p1] * a1).astype(bf),
            blB=_bb(w["sh_bl"][p1] * a1), brB=_bb(w["sh_br"][p1] * a1)))
    rA, profile["A"] = runner(progs["dense1"], in_maps, cfg)
    XL1full = np.concatenate([rA[c]["XL1"] for c in range(cfg.ncores)])
    XL1ch = {f"XL1c{o}": np.ascontiguousarray(
        XL1full[cfg.cbounds[o]:cfg.cbounds[o + 1]])
        for o in range(cfg.nchunk)}
    XR1 = [rA[c]["XR1"] for c in range(cfg.ncores)]

    # ---- launch B: edge1 + dense23 ----------------------------------------
    # w23 rows follow h's permuted hid order; columns carry layer-2/3 att
    # folding (sign-permute + |att| scale) so launch C's tables arrive folded.
    w23lr = np.concatenate([w["mu_Wl"][p1][:, pmu] * amu,
                            w["lv_Wl"][p1][:, plv] * alv,
                            w["mu_Wr"][p1][:, pmu] * amu,
                            w["lv_Wr"][p1][:, plv] * alv], axis=1)
    b23 = np.concatenate([w["mu_bl"][pmu] * amu, w["lv_bl"][plv] * alv,
                          w["mu_br"][pmu] * amu, w["lv_br"][plv] * alv])
    ident = np.eye(P, dtype=np.float32)
    in_maps = []
    for c in range(cfg.ncores):
        in_maps.append(dict(
            XR1=XR1[c], **XL1ch, XL1own=rA[c]["XL1"],
            gidx=pr["gidx16"][c], Od=pr["Od"][c], OTd=pr["OTd"][c],
            inv1B=_bb(inv1), b1B=_bb(w["sh_b"][p1]), identB=ident,
            w23lr=np.ascontiguousarray(w23lr), b23B=_bb(b23)))
    rB, profile["B"] = runner(progs["edge1"], in_maps, cfg)
    XL23full = np.concatenate([rB[c]["XL23"] for c in range(cfg.ncores)])
    XL23ch = {f"XL23c{o}": np.ascontiguousarray(
        XL23full[cfg.cbounds[o]:cfg.cbounds[o + 1]])
        for o in range(cfg.nchunk)}
    XR23 = [rB[c]["XR23"] for c in range(cfg.ncores)]

    # ---- launch C: edge23 --------------------------------------------------
    in_maps = []
    for c in range(cfg.ncores):
        in_maps.append(dict(
            XR23=XR23[c], **XL23ch, XL23own=rB[c]["XL23"],
            gidx=pr["gidx16"][c], Od=pr["Od"][c], OTd=pr["OTd"][c],
            invmuB=_bb(invmu), invlvB=_bb(invlv),
            bmuB=_bb(w["mu_b"][pmu]), blvB=_bb(w["lv_b"][plv])))
    rC, profile["C"] = runner(progs["edge23"], in_maps, cfg)

    MU = np.concatenate([rC[c]["MU"] for c in range(cfg.ncores)])
    LV = np.concatenate([rC[c]["LV"] for c in range(cfg.ncores)])
    mu = np.empty_like(MU)
    lv = np.empty_like(LV)
    mu[:, pmu] = MU          # undo the sign permutation of output columns
    lv[:, plv] = LV
    mu = mu[pr["slot_global"]]
    lv = lv[pr["slot_global"]]
    return (mu, lv), profile


def kernel(**inputs):
    cfg = Cfg()
    x = np.asarray(inputs["x"], np.float32)
    ei = np.asarray(inputs["edge_index"]).astype(np.int64)
    w = {k: np.asarray(v, np.float32) for k, v in inputs.items()
         if k not in ("x", "edge_index")}

    kpos = tuple(int((w[f"{m}_att"].reshape(-1) >= 0).sum())
                 for m in ("sh", "mu", "lv"))
    key = hash(ei.tobytes() + bytes(kpos))
    if _State.key != key:
        pr = prep_graph(cfg, ei)
        _State.prep = pr
        _State.progs = build_progs(cfg, pr, kpos)
        _State.key = key

    trace = bool(int(os.environ.get("GAT_TRACE", "0")))
    runner = functools.partial(_hw_runner, trace=trace)
    (mu, lv), profile = forward(cfg, x, ei, w, _State.prep, _State.progs, runner)
    kernel._last_profile = profile
    return (mu, lv)


kernel._last_profile = None
